# revision 40
# baseline (speedup 1.0000x reference)
"""Two-block single-head transformer (B=4, S=4096, E=256) on 8 TRN2 NeuronCores.

Sharding: core c -> batch b=c//2, query-half h=c%2 (2048 query rows each).
Each core receives its batch's x ROLLED so that its own query rows are always
rows [0:2048] -- this keeps the on-device program identical across cores
(pure SPMD, no partition-id branching).  Attention is permutation-invariant
over keys, so layer-1 may use the rolled key order.  Layer-2 keys come from a
pairwise AllGather of the LN1 outputs in canonical order.

Math per layer (matches torch reference):
  q/k/v = x @ W.T + b ; att = softmax((q k^T)/sqrt(S)) ; o = att v
  layernorm over E with gamma/beta.

fp8 fast path: weights are pre-scaled x16 on the host so q/k/v land in
float8_e4m3 at unit-ish scale.  Scores and att@V run as fp8 DoubleRow
matmuls (K=256 per instruction).  Key chunks are processed in PAIRS: the
two score matmuls of a pair land in one 2-bank PSUM tile and a single
exp instruction covers both.  Exp alternates between the Scalar (ACT)
engine (fp8 out) and the Vector engine, where exp is ONE multiply-add
producing the e4m3 BIT PATTERN of exp(x) (Schraudolph in fp8 space:
bits = round(8*log2(e)*x + 55.65)) written as int8 and bitcast to fp8.
The x16 score scaling folds into the exp scale (1/256); the x16 on V
folds into the softmax denominator by setting the ones-column to 16.

The pair loop is software-pipelined: the scores+exp of pair i+1 are
issued before the att@V of pair i, so the PE never waits on the exp.

V-projection biases are folded on the host (attention rows sum to 1, so
o = att@(v+bv) = att@v + bv): bv1 joins the layer-1 residual, bv2 joins
be1 (with bq2/bk2/"v2" corrected by -bv2 @ W.T), leaving the V copies
as plain ACT Copy ops.

Engine division of labor: PE matmuls; ACT half the exps + all
PSUM->SBUF projection copies (Identity/Copy share the Exp act table: no
table reloads); DVE the other exps + fused (po*recip + resid) +
bn_stats + Newton rsqrt; Pool (gpsimd) LN gamma/beta + collectives.

The layer boundary is pipelined: the AllGather is split into one chunk per
512-query block, and the layer-2 transposes/projections are chunked behind
those, so they overlap the remaining layer-1 attention instead of
serializing after it.
"""

import sys

sys.path.insert(0, "/opt/trn_rl_repo")

import numpy as np
import ml_dtypes

import concourse.bass as bass
import concourse.tile as tile
from concourse import bacc, mybir
from concourse import bass_utils

f32 = mybir.dt.float32
bf16 = mybir.dt.bfloat16
fp8 = mybir.dt.float8e4
i8 = mybir.dt.int8

B, S, E = 4, 4096, 256
P = 128
SQ = S // 2          # query rows per core
DC = E // P          # 2 chunks of the head dim
KC = S // P          # 32 key chunks
NP = KC // 2         # 16 key-chunk pairs (DoubleRow contracts 256 keys)
QB = 512             # query block (matmul moving dim)
NQB = SQ // QB       # 4 query blocks per core
QS = QB // P         # 4 query sub-blocks per block
VR = E + 16          # v tile row stride (pair stride must be %16)
N_CORES = 8
EPS = 1e-5
SCALE = 1.0 / np.sqrt(np.float32(S))
EXP_SCALE = float(SCALE / 256.0)           # x16 on both q and k
EXP_C1 = float(EXP_SCALE * 8.0 * np.log2(np.e))
EXP_C2 = 55.65                              # e4m3 exp bias (Schraudolph)
DR = mybir.MatmulPerfMode.DoubleRow
# per-block pattern: which pair indices exp on ACT (rest on DVE); 11/16 on
# ACT balances DVE's epilogue + copy load.
EXP_ACT = [(i % 8) < 5 for i in range(NP)]
RESID_PRELOAD = False

_COMPILED = None


def _broadcast_ap(vec_ap, parts, n):
    """[n] DRAM vector -> [parts, n] partition-broadcast access pattern."""
    return bass.AP(
        tensor=vec_ap.tensor,
        offset=vec_ap.offset,
        ap=[[0, parts], [1, n]],
    )


def _build():
    nc = bacc.Bacc(
        "TRN2", target_bir_lowering=False, debug=False, num_devices=N_CORES
    )

    # --- kernel I/O (per core) ---
    xT = nc.dram_tensor("xT", [P, DC * S], fp8, kind="ExternalInput").ap()
    xq = nc.dram_tensor("xq", [SQ, E], f32, kind="ExternalInput").ap()
    wts = {
        n: nc.dram_tensor(
            n, [P, DC * E], fp8 if n.endswith("1") else bf16,
            kind="ExternalInput",
        ).ap()
        for n in ["wqt1", "wkt1", "wvt1", "wqt2", "wkt2", "wvt2"]
    }
    vecs = {
        n: nc.dram_tensor(n, [E], f32, kind="ExternalInput").ap()
        for n in ["bq1", "bk1", "bq2", "bk2", "g1", "be1", "g2", "be2"]
    }
    y = nc.dram_tensor("y", [SQ, E], f32, kind="ExternalOutput").ap()

    with tile.TileContext(nc) as tc:
        _emit(nc, tc, xT, xq, wts, vecs, y)

    nc.compile()
    return nc


def _emit(nc, tc, xT, xq, wts, vecs, y):
    from contextlib import ExitStack

    Ident = mybir.ActivationFunctionType.Identity
    Copy = mybir.ActivationFunctionType.Copy
    Exp = mybir.ActivationFunctionType.Exp
    mult = mybir.AluOpType.mult
    add = mybir.AluOpType.add
    subtract = mybir.AluOpType.subtract

    ctx = ExitStack()
    with ctx:
        const = ctx.enter_context(tc.tile_pool(name="const", bufs=1))
        srcT_pool = ctx.enter_context(tc.tile_pool(name="srcT", bufs=1))
        kt_pool = ctx.enter_context(tc.tile_pool(name="kt", bufs=2))
        v_pool = ctx.enter_context(tc.tile_pool(name="v", bufs=2))
        qt_pool = ctx.enter_context(tc.tile_pool(name="qt", bufs=2))
        o_pool = ctx.enter_context(tc.tile_pool(name="okeep", bufs=16))
        resid_pool = ctx.enter_context(tc.tile_pool(name="resid", bufs=16))
        work = ctx.enter_context(tc.tile_pool(name="work", bufs=4))
        expp = ctx.enter_context(tc.tile_pool(name="expp", bufs=4))
        stats = ctx.enter_context(tc.tile_pool(name="stats", bufs=8))
        dram = ctx.enter_context(tc.tile_pool(name="dram", bufs=1, space="DRAM"))
        # 2 two-bank score-pair slots (shared with projections) + 4 po banks
        mm_ps = ctx.enter_context(tc.tile_pool(name="mm_ps", bufs=2, space="PSUM"))
        o_ps = ctx.enter_context(tc.tile_pool(name="o_ps", bufs=4, space="PSUM"))

        # --- constants (sync HWDGE queue, in order of first use) ---
        w_sb, bias_sb, bcast_sb = {}, {}, {}

        def _load_w(n):
            dt = fp8 if n.endswith("1") else bf16
            t = const.tile([P, DC, E], dt, tag=f"w_{n}", name=f"w_{n}")
            nc.sync.dma_start(
                out=t[:], in_=wts[n].rearrange("p (dc o) -> p dc o", dc=DC)
            )
            w_sb[n] = t

        def _load_b(n):
            t = const.tile([P, DC], f32, tag=f"b_{n}", name=f"b_{n}")
            nc.sync.dma_start(
                out=t[:], in_=vecs[n].rearrange("(dc p) -> p dc", p=P)
            )
            bias_sb[n] = t

        def _load_bc(n):
            t = const.tile([P, E], f32, tag=f"bc_{n}", name=f"bc_{n}")
            nc.sync.dma_start(out=t[:], in_=_broadcast_ap(vecs[n], P, E))
            bcast_sb[n] = t

        # need-ordered on the sync queue: layer-1 K path first, then the
        # rest behind the first xT chunks (emitted in the layer-1 section).
        for n in ["wkt1", "wvt1", "wqt1"]:
            _load_w(n)
        _load_b("bk1")
        _load_b("bq1")

        def _load_late_consts():
            for n in ["g1", "be1"]:
                _load_bc(n)
            for n in ["wqt2", "wkt2", "wvt2"]:
                _load_w(n)
            for n in ["bq2", "bk2"]:
                _load_b(n)
            for n in ["g2", "be2"]:
                _load_bc(n)

        o_bounce = dram.tile([SQ, E], bf16)
        # one contiguous AllGather output per query block:
        # rows [0:QB] = own-half canonical rows qb*QB..,
        # rows [QB:2QB] = other-half canonical rows SQ+qb*QB..
        o_chunks = [
            dram.tile([2 * QB, E], bf16, name=f"agchunk{i}") for i in range(NQB)
        ]

        def proj_k_chunk(kT_sb, srcT_sb, wk, bk, dst_c, src_c, dr=False):
            """KT[:, :, dst_c*QB:+QB] (fp8) from srcT columns [src_c*QB:+QB].

            Matmuls into the two banks of one pair slot (fp8 DoubleRow when
            dr=True, layer 1); DVE adds the per-partition bias, writes fp8.
            """
            ps = mm_ps.tile([P, 2, QB], f32, tag="mm", name="pk")
            for oc in range(DC):
                if dr:
                    nc.tensor.matmul(
                        ps[:, oc, :],
                        lhsT=wk[:, :, oc * P:(oc + 1) * P],
                        rhs=srcT_sb[:, :, src_c * QB:(src_c + 1) * QB],
                        start=True, stop=True, perf_mode=DR,
                    )
                else:
                    for dc in range(DC):
                        nc.tensor.matmul(
                            ps[:, oc, :],
                            lhsT=wk[:, dc, oc * P:(oc + 1) * P],
                            rhs=srcT_sb[:, dc, src_c * QB:(src_c + 1) * QB],
                            start=(dc == 0),
                            stop=(dc == DC - 1),
                        )
            for oc in range(DC):
                nc.vector.tensor_scalar_add(
                    out=kT_sb[:, oc, dst_c * QB:(dst_c + 1) * QB],
                    in0=ps[:, oc, :], scalar1=bk[:, oc:oc + 1],
                )

        def proj_v_chunk(v_sb, srcT_sb, wv, dst_c, src_c, dr=False):
            """V rows [dst_c*QB : +QB] (4 sub-chunks of 128, fp8) from srcT.

            Bias-free (folded into residuals on the host): plain ACT Copy.
            """
            for half in range(2):
                ps = mm_ps.tile([P, 2, QB], f32, tag="mm", name="pv")
                for i in range(2):
                    sc_dst = dst_c * (QB // P) + half * 2 + i
                    sc_src = src_c * (QB // P) + half * 2 + i
                    if dr:
                        nc.tensor.matmul(
                            ps[:, i, :E],
                            lhsT=srcT_sb[:, :, sc_src * P:(sc_src + 1) * P],
                            rhs=wv[:, :, :],
                            start=True, stop=True, perf_mode=DR,
                        )
                    else:
                        for dc in range(DC):
                            nc.tensor.matmul(
                                ps[:, i, :E],
                                lhsT=srcT_sb[:, dc, sc_src * P:(sc_src + 1) * P],
                                rhs=wv[:, dc, :],
                                start=(dc == 0),
                                stop=(dc == DC - 1),
                            )
                sc_dst = dst_c * (QB // P) + half * 2
                nc.scalar.activation(
                    out=v_sb[:, sc_dst:sc_dst + 2, :E], in_=ps[:, :, :E],
                    func=Copy,
                )

        def proj_q_chunk(qT_out, srcT_sb, wq, bq, qc, dr=False):
            ps = mm_ps.tile([P, 2, QB], f32, tag="mm", name="pq")
            for oc in range(DC):
                if dr:
                    nc.tensor.matmul(
                        ps[:, oc, :],
                        lhsT=wq[:, :, oc * P:(oc + 1) * P],
                        rhs=srcT_sb[:, :, qc * QB:(qc + 1) * QB],
                        start=True, stop=True, perf_mode=DR,
                    )
                else:
                    for dc in range(DC):
                        nc.tensor.matmul(
                            ps[:, oc, :],
                            lhsT=wq[:, dc, oc * P:(oc + 1) * P],
                            rhs=srcT_sb[:, dc, qc * QB:(qc + 1) * QB],
                            start=(dc == 0),
                            stop=(dc == DC - 1),
                        )
            for oc in range(DC):
                nc.scalar.activation(
                    out=qT_out[:, oc, qc * QB:(qc + 1) * QB],
                    in_=ps[:, oc, :], func=Ident, bias=bq[:, oc:oc + 1],
                )

        def attention_block(kT_sb, v_sb, qT_sb, qb, resid_tiles, g_bc, be_bc,
                            out_cb, pair_order=None, mid_emit=None,
                            dst_alloc=None):
            """One 512-query attention block + residual + layernorm.

            Scores and att@V are fp8 DoubleRow matmuls over key-chunk PAIRS
            (256 keys contracted per instruction), software-pipelined so the
            exp of pair i overlaps the att@V of pair i-1.
            """
            if pair_order is None:
                pair_order = list(range(NP))
            po = [
                o_ps.tile([P, E + 1], f32, tag="ops", name=f"po{i}")
                for i in range(QS)
            ]
            resid = [f() for f in resid_tiles]

            state = {}

            def scores_pair(i):
                pr = pair_order[i]
                ex = expp.tile([P, 2, QB], fp8, tag="exp")
                ps = mm_ps.tile([P, 2, QB], f32, tag="mm")
                for m in range(2):
                    kc = 2 * pr + m
                    nc.tensor.matmul(
                        ps[:, m, :],
                        lhsT=kT_sb[:, :, kc * P:(kc + 1) * P],
                        rhs=qT_sb[:, :, qb * QB:(qb + 1) * QB],
                        start=True, stop=True, perf_mode=DR,
                    )
                if EXP_ACT[i]:
                    ins = nc.scalar.activation(
                        out=ex[:], in_=ps[:], func=Exp, scale=EXP_SCALE,
                    )
                else:
                    ins = nc.vector.tensor_scalar(
                        out=ex[:].bitcast(i8), in0=ps[:],
                        scalar1=EXP_C1, scalar2=EXP_C2,
                        op0=mult, op1=add,
                    )
                state[i] = (ex, ins)

            scores_pair(0)
            for i in range(NP):
                if mid_emit and i in mid_emit:
                    mid_emit[i](state[i][1])
                if i + 1 < NP:
                    scores_pair(i + 1)
                ex, _ = state.pop(i)
                pr = pair_order[i]
                for qs in range(QS):
                    nc.tensor.matmul(
                        po[qs][:],
                        lhsT=ex[:, :, qs * P:(qs + 1) * P],
                        rhs=v_sb[:, 2 * pr:2 * pr + 2, :E + 1],
                        start=(i == 0),
                        stop=(i == NP - 1),
                        perf_mode=DR,
                    )
            ats, mvs = [], []
            var4 = stats.tile([P, QS], f32, tag="var4")
            for qs in range(QS):
                den = stats.tile([P, 1], f32, tag="den")
                nc.vector.reciprocal(out=den[:], in_=po[qs][:, E:E + 1])
                at = work.tile([P, E], f32, tag="attn", name=f"at{qs}")
                nc.vector.scalar_tensor_tensor(
                    out=at[:], in0=po[qs][:, :E], scalar=den[:],
                    in1=resid[qs][:], op0=mult, op1=add,
                )
                st = stats.tile([P, nc.vector.BN_STATS_DIM], f32, tag="bst")
                nc.vector.bn_stats(out=st[:], in_=at[:])
                mv = stats.tile([P, nc.vector.BN_AGGR_DIM], f32, tag="bag",
                                name=f"mv{qs}")
                nc.vector.bn_aggr(out=mv[:], in_=st[:])
                nc.vector.tensor_scalar_add(
                    out=var4[:, qs:qs + 1], in0=mv[:, 1:2], scalar1=EPS
                )
                ats.append(at)
                mvs.append(mv)
            # rstd = rsqrt(var4) on DVE: seed 1.5 - 0.5v (clamped), 3 Newton
            # iterations.  var is ~1 +- 0.1 for LN of resid-dominated rows,
            # so this converges to ~1e-7 without touching the ACT table.
            rstd = stats.tile([P, QS], f32, tag="rstd")
            tmp = stats.tile([P, QS], f32, tag="nwt")
            nc.vector.tensor_scalar(
                out=rstd[:], in0=var4[:], scalar1=-0.5, scalar2=1.5,
                op0=mult, op1=add,
            )
            nc.vector.tensor_scalar_max(out=rstd[:], in0=rstd[:], scalar1=0.35)
            for _ in range(3):
                nc.vector.tensor_mul(out=tmp[:], in0=rstd[:], in1=rstd[:])
                nc.vector.tensor_mul(out=tmp[:], in0=tmp[:], in1=var4[:])
                nc.vector.tensor_scalar(
                    out=tmp[:], in0=tmp[:], scalar1=-0.5, scalar2=1.5,
                    op0=mult, op1=add,
                )
                nc.vector.tensor_mul(out=rstd[:], in0=rstd[:], in1=tmp[:])
            mid_ins = None
            for qs in range(QS):
                at = ats[qs]
                nc.vector.tensor_scalar(
                    out=at[:], in0=at[:],
                    scalar1=mvs[qs][:, 0:1], scalar2=rstd[:, qs:qs + 1],
                    op0=subtract, op1=mult,
                )
                nc.gpsimd.tensor_mul(out=at[:], in0=at[:], in1=g_bc[:])
                dst = dst_alloc() if dst_alloc else at
                ins = nc.gpsimd.tensor_add(out=dst[:], in0=at[:], in1=be_bc[:])
                if qs == 1:
                    mid_ins = ins
                out_cb(qs, dst)
            return mid_ins

        # ---------------- layer 1 ----------------
        xT_sb = srcT_pool.tile([P, DC, S], fp8, tag="srcT")
        kT1 = kt_pool.tile([P, DC, S], fp8, tag="kt")
        v1 = v_pool.tile([P, KC, VR], fp8, tag="v")
        nc.vector.memset(v1[:, :, E:E + 1], 16.0)
        qT1 = qt_pool.tile([P, DC, SQ], fp8, tag="qt")
        xT_r = xT.rearrange("p (dc s) -> p dc s", dc=DC)
        # all 16 layer-1 residual tiles are preloaded during the projection
        # phase so no DMA for them ever sits in the sync queue mid-attention
        # (where it could queue behind an AllGather-gated transpose).
        resid1 = [None] * (NQB * QS)

        def _load_resids(lo, hi):
            for i in range(lo, hi):
                t = resid_pool.tile([P, E], f32, tag="xq", name=f"xq{i}")
                nc.sync.dma_start(out=t[:], in_=xq[i * P:(i + 1) * P, :])
                resid1[i] = t

        G = 1024
        for c in range(S // QB):
            if c % 2 == 0:
                g = c // 2
                nc.sync.dma_start(
                    out=xT_sb[:, :, g * G:(g + 1) * G],
                    in_=xT_r[:, :, g * G:(g + 1) * G],
                )
            if c == 2:
                _load_late_consts()
            if c == 3 and RESID_PRELOAD:
                _load_resids(0, 8)
            elif c == 5 and RESID_PRELOAD:
                _load_resids(8, 16)
            proj_k_chunk(kT1, xT_sb, w_sb["wkt1"], bias_sb["bk1"], c, c, dr=True)
            proj_v_chunk(v1, xT_sb, w_sb["wvt1"], c, c, dr=True)
            if c < NQB:
                proj_q_chunk(qT1, xT_sb, w_sb["wqt1"], bias_sb["bq1"], c,
                             dr=True)

        ob_tiles = []
        anchors = {}

        def emit_l2_transposes(qb):
            """Emitted mid-block so they sit in the sync queue BEFORE the
            enclosing block's epilogue o_bounce writes; by then the chunk's
            AllGather has landed and they drain immediately."""
            for dc in range(DC):
                nc.sync.dma_start_transpose(
                    out=oqT_sb[:, dc, qb * QB:(qb + 1) * QB],
                    in_=o_bounce[qb * QB:(qb + 1) * QB, dc * P:(dc + 1) * P],
                )
            for half in range(2):
                r0 = half * SQ + qb * QB
                for dc in range(DC):
                    nc.sync.dma_start_transpose(
                        out=oT_sb[:, dc, r0:r0 + QB],
                        in_=o_chunks[qb][half * QB:(half + 1) * QB,
                                         dc * P:(dc + 1) * P],
                    )

        def emit_l2_projs(qb):
            proj_q_chunk(qT2, oqT_sb, w_sb["wqt2"], bias_sb["bq2"], qb)
            for half in range(2):
                c = half * NQB + qb
                proj_k_chunk(kT2, oT_sb, w_sb["wkt2"], bias_sb["bk2"], c, c)
                proj_v_chunk(v2, oT_sb, w_sb["wvt2"], c, c)

        def emit_l2_chunk(qb):
            emit_l2_transposes(qb)
            emit_l2_projs(qb)

        def make_resid1(qb, qs):
            if RESID_PRELOAD:
                return lambda: resid1[qb * QS + qs]
            def f():
                t = work.tile([P, E], f32, tag="xq")
                nc.sync.dma_start(
                    out=t[:], in_=xq[(qb * QS + qs) * P:(qb * QS + qs + 1) * P, :]
                )
                return t
            return f

        # layer-2 destination tiles (written chunk-by-chunk as AllGather
        # results land, interleaved with the remaining layer-1 attention)
        oT_sb = srcT_pool.tile([P, DC, S], bf16, tag="oT")
        oqT_sb = qt_pool.tile([P, DC, SQ], bf16, tag="oqT")
        kT2 = kt_pool.tile([P, DC, S], fp8, tag="kt")
        v2 = v_pool.tile([P, KC, VR], fp8, tag="v")
        nc.vector.memset(v2[:, :, E:E + 1], 16.0)
        qT2 = qt_pool.tile([P, DC, SQ], fp8, tag="qt")

        for qb in range(NQB):
            def out1(qs, ot, qb=qb):
                r = (qb * QS + qs) * P
                nc.sync.dma_start(out=o_bounce[r:r + P, :], in_=ot[:])

            def dst1():
                ot = o_pool.tile([P, E], bf16, tag="okeep")
                ob_tiles.append(ot)
                return ot

            anchors[qb] = attention_block(
                kT1, v1, qT1, qb,
                [make_resid1(qb, qs) for qs in range(QS)],
                bcast_sb["g1"], bcast_sb["be1"], out1, dst_alloc=dst1,
                mid_emit=(
                    {10: (lambda a, qb=qb: emit_l2_transposes(qb - 2))}
                    if qb >= 2 else None
                ),
            )

            # exchange this block's LN1 rows within the batch pair.
            nc.gpsimd.collective_compute(
                "AllGather",
                mybir.AluOpType.bypass,
                ins=[o_bounce[qb * QB:(qb + 1) * QB, :].opt()],
                outs=[o_chunks[qb].opt()],
                replica_groups=[[0, 1], [2, 3], [4, 5], [6, 7]],
            )

            # layer-2 chunk work TWO blocks behind: with fp8 blocks running
            # ~2x faster than the AllGather round-trip, one block of slack is
            # not enough and the PE stream would head-of-line stall on the
            # transposes (already emitted mid-block above).
            if qb >= 2:
                emit_l2_projs(qb - 2)

        # remaining layer-2 chunk work: chunk 2 between the layers, chunk 3
        # mid-way through layer-2 block 0 (its AllGather fires at the end of
        # layer 1 and needs ~15us to land).
        emit_l2_chunk(NQB - 2)

        # key-chunk pairs ordered by AllGather-chunk arrival:
        # cc chunks {0,4},{1,5},{2,6},{3,7} -> pairs (2c, 2c+1)
        pair_order2 = [
            p
            for cc in [0, 4, 1, 5, 2, 6, 3, 7]
            for p in (2 * cc, 2 * cc + 1)
        ]
        for qb in range(NQB):
            def out2(qs, at, qb=qb):
                r = (qb * QS + qs) * P
                nc.sync.dma_start(out=y[r:r + P, :], in_=at[:])

            attention_block(
                kT2, v2, qT2, qb,
                [
                    (lambda qs=qs, qb=qb: ob_tiles[qb * QS + qs])
                    for qs in range(QS)
                ],
                bcast_sb["g2"], bcast_sb["be2"], out2,
                pair_order=pair_order2,
                mid_emit=(
                    {8: (lambda a: emit_l2_chunk(NQB - 1))}
                    if qb == 0 else None
                ),
            )


def _prep_inputs(x, Wq1, bq1, Wk1, bk1, Wv1, bv1, Wq2, bq2, Wk2, bk2, Wv2,
                 bv2, g1, beta1, g2, beta2):
    bfl = ml_dtypes.bfloat16
    f = np.float32
    shared = {}
    def _stripe(a2d):
        e_in, n = a2d.shape
        return np.ascontiguousarray(
            a2d.reshape(DC, P, n).transpose(1, 0, 2).reshape(P, DC * n)
        )

    # x16 pre-scaling puts q/k/v at unit-ish scale for fp8 quantization;
    # the exp scale (1/256) and the 16.0 ones-column undo it exactly.
    # Layer-1 weights ship as fp8 (DoubleRow projections from fp8 xT);
    # layer-2 weights stay bf16 (their source oT comes via bf16 transposes).
    f8l = ml_dtypes.float8_e4m3
    for n, w in [("wqt1", Wq1), ("wkt1", Wk1), ("wvt1", Wv1),
                 ("wqt2", Wq2), ("wkt2", Wk2), ("wvt2", Wv2)]:
        dt = f8l if n.endswith("1") else bfl
        shared[n] = _stripe((16.0 * np.asarray(w, f).T).astype(dt))
    # V-biases are folded into the residual path (attention rows sum to 1,
    # so att@(v + bv) = att@v + bv), leaving both V projections bias-free:
    #   layer 1: LN1_in = att1@(x Wv1.T) + x + bv1  -> bv1 joins resid1 (xq);
    #            the projections read x unchanged, so no correction needed.
    #   layer 2: resid2 (= LN1 output tiles = AllGather payload) carries
    #            o1' = o1 + shift via be1 += shift.  Then
    #            LN2_in' = att2@(o1 Wv2.T) + o1 + shift@Wv2.T + shift, so
    #            shift @ (Wv2.T + I) = bv2 makes it exact; q2/k2 biases are
    #            corrected by -shift @ W.T to cancel the shifted source.
    bv1 = np.asarray(bv1, f)
    bv2 = np.asarray(bv2, f)
    if np.any(bv2):
        shift = np.linalg.solve(np.asarray(Wv2, f) + np.eye(E, dtype=f), bv2)
    else:
        shift = np.zeros(E, f)
    bq2c = np.asarray(bq2, f) - shift @ np.asarray(Wq2, f).T
    bk2c = np.asarray(bk2, f) - shift @ np.asarray(Wk2, f).T
    for n, v in [("bq1", bq1), ("bk1", bk1), ("bq2", bq2c), ("bk2", bk2c)]:
        shared[n] = np.ascontiguousarray(16.0 * np.asarray(v, f))
    shared["g1"] = np.ascontiguousarray(np.asarray(g1, f))
    shared["be1"] = np.ascontiguousarray(np.asarray(beta1, f) + shift)
    shared["g2"] = np.ascontiguousarray(np.asarray(g2, f))
    shared["be2"] = np.ascontiguousarray(np.asarray(beta2, f))

    x = np.asarray(x, f)
    in_maps = []
    for c in range(N_CORES):
        b, h = c // 2, c % 2
        xb = x[b]
        if h:
            xb = np.concatenate([xb[SQ:], xb[:SQ]], axis=0)
        m = dict(shared)
        m["xT"] = _stripe(np.ascontiguousarray(xb.T).astype(f8l))
        m["xq"] = np.ascontiguousarray(xb[:SQ] + bv1)
        in_maps.append(m)
    return in_maps


def _get_compiled():
    global _COMPILED
    if _COMPILED is None:
        _COMPILED = _build()
    return _COMPILED


def run(trace=False, **inputs):
    nc = _get_compiled()
    in_maps = _prep_inputs(**inputs)
    last_err = None
    for _ in range(3):
        try:
            res = bass_utils.run_bass_kernel_spmd(
                nc, in_maps, core_ids=list(range(N_CORES)), trace=trace
            )
            break
        except Exception as e:  # transient NRT device errors; retry
            last_err = e
    else:
        raise last_err
    out = np.empty((B, S, E), np.float32)
    for c in range(N_CORES):
        b, h = c // 2, c % 2
        out[b, h * SQ:(h + 1) * SQ] = res.results[c]["y"]
    return out, res


def kernel(**inputs):
    out, _ = run(trace=False, **inputs)
    return out
